# revision 64
# baseline (speedup 1.0000x reference)
"""GQA attention kernel for 8 TRN2 NeuronCores (Bass/Tile, SPMD).

Sharding: core c -> (batch b = c // 4, kv-head kv = c % 4). Each core computes
the 4 query heads of its kv group for its batch and a partial (transposed)
output projection; the host sums the 4 partials per batch.

All matmul inputs are bf16 (the PE streams 1 col/cycle at any free size,
DMA and LDWEIGHTS bytes halve, and everything stays SBUF-resident — no pool
juggling). Softmax denominators accumulate on the vector engine in fp16
(acc += exp tile) with a single ones-matmul per head instead of a
per-k-tile ones-matmul chain; PV sums are evicted unnormalized and scaled
in place once the broadcast 1/l lands. Heads run in pairs sharing one
2-bank psum tile and a single exp instruction per k-tile, halving ACT
instruction count in the exp-bound inner loop. The whole kernel is software
pipelined: the next chunk's projection chains and the previous chunk's
output-projection groups are paced evenly through the attention kt-loop, so
the PE always has matmul work while the ACT engine drains the exp backlog,
and output DMA spreads across the run instead of piling up in a tail.
"""

import os
import sys

import numpy as np

for _p in ("/opt/trn_rl_repo", "/root/.axon_site/_ro/trn_rl_repo"):
    if os.path.isdir(_p) and _p not in sys.path:
        sys.path.insert(0, _p)

import ml_dtypes  # noqa: E402

import concourse.bass as bass  # noqa: E402
import concourse.mybir as mybir  # noqa: E402
from concourse import bacc  # noqa: E402
from concourse.tile import TileContext  # noqa: E402
from concourse.bass_utils import run_bass_kernel_spmd  # noqa: E402

B, T, D = 2, 2048, 2048
H, HKV, HD = 16, 4, 128
G = H // HKV            # query heads per kv head (= per core)
EQ = G * HD             # 512: query-projection rows per core
P = 128
TC = 512                # t-chunk (free dim of most matmuls)
NJ = T // TC            # 4 chunks
DT = D // P             # 16 contraction tiles
SCALE = 1.0 / float(np.sqrt(HD))

F32 = mybir.dt.float32
F32R = mybir.dt.float32r
BF16 = mybir.dt.bfloat16
F16 = mybir.dt.float16
EXP = mybir.ActivationFunctionType.Exp

_CACHE = {}


def _build():
    nc = bacc.Bacc("TRN2", target_bir_lowering=False, debug=False)

    # All inputs arrive pre-transposed into SBUF layout (partition dim first,
    # contiguous per partition) so every DMA runs at full descriptor rate.
    xT = nc.declare_dram_parameter("xT", [P, NJ, 4, 4, TC], BF16, isOutput=False)
    wqT = nc.declare_dram_parameter("wqT", [P, G, DT, HD], BF16, isOutput=False)
    wkT = nc.declare_dram_parameter("wkT", [P, DT, HD], BF16, isOutput=False)
    wvT = nc.declare_dram_parameter("wvT", [P, DT, HD], BF16, isOutput=False)
    woT = nc.declare_dram_parameter("woT", [P, G, D], BF16, isOutput=False)
    cosT = nc.declare_dram_parameter("cosT", [HD, T], BF16, isOutput=False)
    sinT = nc.declare_dram_parameter("sinT", [HD, T], BF16, isOutput=False)
    rmat = nc.declare_dram_parameter("rmat", [HD, HD], BF16, isOutput=False)
    iden = nc.declare_dram_parameter("iden", [P, P], BF16, isOutput=False)
    masks = nc.declare_dram_parameter("masks", [P, G, TC], BF16, isOutput=False)
    ones_k = nc.declare_dram_parameter("ones_k", [P, 1], F16, isOutput=False)
    yT = nc.declare_dram_parameter("yT", [NJ, D, TC], BF16, isOutput=True)

    with TileContext(nc) as tc:
        with (
            tc.tile_pool(name="const", bufs=1) as cst,
            tc.tile_pool(name="wts", bufs=1) as wts,
            tc.tile_pool(name="xs", bufs=1) as xs,
            tc.tile_pool(name="kv", bufs=1) as kvp,
            tc.tile_pool(name="qk", bufs=2) as qk,
            tc.tile_pool(name="vt", bufs=2) as vtp,
            tc.tile_pool(name="rtmp", bufs=2) as rtmp,
            tc.tile_pool(name="work", bufs=8) as wkp,
            tc.tile_pool(name="small", bufs=2) as sml,
            tc.tile_pool(name="yout", bufs=4) as yop,
            tc.tile_pool(name="ps_o", bufs=2, space="PSUM") as ps_o,
            tc.tile_pool(name="ps_s", bufs=2, space="PSUM") as ps_s,
            tc.tile_pool(name="ps_a", bufs=2, space="PSUM") as ps_a,
        ):
            # Constants ride the gpsimd SWDGE ring so they don't delay the
            # weight/x loads on the two HWDGE rings.
            cos_sb = cst.tile([HD, T], BF16, tag="cos")
            sin_sb = cst.tile([HD, T], BF16, tag="sin")
            rmat_sb = cst.tile([HD, HD], BF16, tag="rmat")
            iden_sb = cst.tile([P, P], BF16, tag="iden")
            mask_sb = cst.tile([P, G, TC], BF16, tag="mask")
            onek_sb = cst.tile([P, 1], F16, tag="onek")
            nc.gpsimd.dma_start(cos_sb[:], cosT[:])
            nc.gpsimd.dma_start(sin_sb[:], sinT[:])
            nc.gpsimd.dma_start(rmat_sb[:], rmat[:])
            nc.gpsimd.dma_start(iden_sb[:], iden[:])
            nc.gpsimd.dma_start(mask_sb[:], masks[:])
            nc.gpsimd.dma_start(onek_sb[:], ones_k[:])

            # Weights ride the scalar HWDGE ring in first-use order; x rides
            # the sync ring, quarters in consumption order. Everything is
            # resident for the whole kernel (bf16 halves the footprint).
            wq_sb = wts.tile([P, G, DT, HD], BF16, tag="wq")
            wk_sb = wts.tile([P, DT, HD], BF16, tag="wk")
            wv_sb = wts.tile([P, DT, HD], BF16, tag="wv")
            wo_sb = wts.tile([P, G, D], BF16, tag="wo")
            xq_sb = {}
            for j in range(NJ):
                for q in range(4):
                    xq_sb[(j, q)] = xs.tile([P, 4, TC], BF16,
                                            tag=f"x{j}q{q}", name=f"x{j}q{q}")

            # Both HWDGE rings share one per-core HBM bandwidth pool, so the
            # load order across BOTH rings must match consumption order:
            # (wv slice, its x j0 dt-slices) pairs, wk, per-head wq — and
            # only then the later chunks' x and wo, which otherwise steal
            # bandwidth from the weight stream exactly while the Q chains
            # starve. Slices alternate rings to keep delivery in order.
            seq = []
            for q in range(4):
                seq.append((wv_sb[:, 4 * q:4 * q + 4],
                            wvT[:, 4 * q:4 * q + 4]))
                for dq in range(4):
                    seq.append((xq_sb[(0, q)][:, dq:dq + 1],
                                xT[:, 0, q, dq:dq + 1]))
            for q in range(4):
                seq.append((wk_sb[:, 4 * q:4 * q + 4],
                            wkT[:, 4 * q:4 * q + 4]))
            for g in range(G):
                seq.append((wq_sb[:, g], wqT[:, g]))
            for q in range(4):
                # x j1 lands right as B_0's striped A_1 chains consume it;
                # per-dt slices keep them paced by arrival, not completion.
                for dq in range(4):
                    seq.append((xq_sb[(1, q)][:, dq:dq + 1],
                                xT[:, 1, q, dq:dq + 1]))
            for j in range(2, NJ):
                for q in range(4):
                    seq.append((xq_sb[(j, q)][:], xT[:, j, q]))
            for g in range(G):
                seq.append((wo_sb[:, g], woT[:, g]))
            for i, (dst, srcp) in enumerate(seq):
                (nc.sync if i % 2 == 0 else nc.scalar).dma_start(dst, srcp)

            kt_sb = kvp.tile([HD, T], BF16, tag="kt")
            v_sb = kvp.tile([P, DT, HD], BF16, tag="v")
            otn = kvp.tile([HD, G, T], BF16, tag="otn")
            acc_sb = kvp.tile([P, G, TC], F16, tag="acc")

            OFFS = {0: 0, 1: 128, 2: 256, 3: 384}
            DEPTH = 4  # exp/mask run four S-tiles ahead of PV

            qt_sb = {}

            def a_thunks(j):
                """Projections of t-chunk j + RoPE + V transpose, as a list
                of emission thunks (one per chain + a flush) so they can be
                striped into the previous attention block. Each chain's RoPE
                matmul is emitted one chain later so its eviction + cos-mul
                hide under the next chain's matmuls."""
                jsl = slice(j * TC, (j + 1) * TC)
                qt = qk.tile([HD, G, TC], BF16, tag="qt", name="qt")
                qt_sb[j] = qt
                vt = vtp.tile([HD, TC], BF16, tag="vt", name="vt")
                rope_q = []

                def finish_rope(s, t1):
                    # s <- s*cos + rotate_half(s)*sin; t1 = s*cos precomputed
                    pr = ps_s.tile([HD, TC], F32, tag="s", name="pr")
                    nc.tensor.matmul(pr[:], rmat_sb[:], s, start=True,
                                     stop=True)
                    nc.vector.tensor_mul(out=s, in0=pr[:], in1=sin_sb[:, jsl])
                    nc.vector.tensor_add(out=s, in0=s, in1=t1[:])

                def chain(a):
                    acc = ps_a.tile([P, TC], F32, tag="a", name="acc")
                    for dt in range(DT):
                        if a == 0:
                            lhsT = wv_sb[:, dt]
                        elif a == 1:
                            lhsT = wk_sb[:, dt]
                        else:
                            lhsT = wq_sb[:, a - 2, dt]
                        nc.tensor.matmul(acc[:], lhsT,
                                         xq_sb[(j, dt // 4)][:, dt % 4],
                                         start=(dt == 0), stop=(dt == DT - 1))
                    if a == 0:
                        nc.vector.tensor_copy(vt[:], acc[:])
                    else:
                        s = kt_sb[:, jsl] if a == 1 else qt[:, a - 2]
                        nc.vector.tensor_copy(s, acc[:])
                        t1 = rtmp.tile([HD, TC], BF16, tag="t1", name="t1")
                        nc.vector.tensor_mul(out=t1[:], in0=s,
                                             in1=cos_sb[:, jsl])
                        rope_q.append((s, t1))
                    if a == 1:
                        # V transpose rides the PE while K's eviction drains.
                        for tt in range(NJ):
                            pvt = ps_s.tile([P, P], BF16, tag="s", name="pvt")
                            nc.tensor.transpose(pvt[:],
                                                vt[:, tt * P:(tt + 1) * P],
                                                iden_sb[:])
                            nc.vector.tensor_copy(v_sb[:, NJ * j + tt],
                                                  pvt[:])
                    if len(rope_q) >= 2:
                        finish_rope(*rope_q.pop(0))

                def flush():
                    while rope_q:
                        finish_rope(*rope_q.pop(0))

                return [lambda a=a: chain(a) for a in range(6)] + [flush]

            pending_norm = []

            def do_norm():
                # otn holds the unnormalized PV sum; scale it by 1/l in
                # place once the broadcast lands.
                h, jsl, rinv = pending_norm.pop(0)
                binv = sml.tile([P, TC], F32, tag="binv", name="binv")
                nc.gpsimd.partition_broadcast(binv[:], rinv[:])
                nc.vector.tensor_mul(out=otn[:, h, jsl], in0=otn[:, h, jsl],
                                     in1=binv[:])

            def c_group(j, dt):
                """One output-projection dt-group for t-chunk j; copies
                alternate DVE/ACT and DMA issues alternate sync/scalar so no
                single engine paces the chain."""
                jsl = slice(j * TC, (j + 1) * TC)
                py = ps_a.tile([P, TC], F32, tag="a", name="py")
                for g in range(G):
                    nc.tensor.matmul(py[:],
                                     wo_sb[:, g, dt * P:(dt + 1) * P],
                                     otn[:, g, jsl],
                                     start=(g == 0), stop=(g == G - 1))
                y_sb = yop.tile([P, TC], BF16, tag="ysb", name="ysb")
                if dt % 2 == 0:
                    nc.scalar.copy(y_sb[:], py[:])
                    nc.sync.dma_start(yT[j, dt * P:(dt + 1) * P, :], y_sb[:])
                else:
                    nc.vector.tensor_copy(y_sb[:], py[:])
                    nc.scalar.dma_start(yT[j, dt * P:(dt + 1) * P, :],
                                        y_sb[:])

            def b_phase(j, filler):
                """Attention for q-block j, all 4 heads, causal. `filler` is
                a list of emission thunks (next chunk's projections, previous
                chunk's output groups) paced evenly through the kt loop so
                the PE always has matmul work while the ACT engine drains the
                exp backlog."""
                jsl = slice(j * TC, (j + 1) * TC)
                qt = qt_sb[j]
                nk = 4 * (j + 1)
                n_it = (G // 2) * nk
                it = 0
                filled = 0

                def pace():
                    nonlocal filled
                    want = (len(filler) * it) // n_it
                    while filled < want:
                        filler[filled]()
                        filled += 1

                pending_ones = []

                def do_ones():
                    # Softmax denominator: one ones-matmul over the DVE-
                    # accumulated exp sums. Deferred into the next head pair
                    # so the PE isn't stalled on the exp->accumulate chain
                    # at the boundary.
                    h = pending_ones.pop(0)
                    pl = ps_a.tile([1, TC], F32, tag="a", name="pl")
                    nc.tensor.matmul(pl[:], onek_sb[:], acc_sb[:, h],
                                     start=True, stop=True)
                    rinv = sml.tile([1, TC], F32, tag="rinv", name="rinv")
                    nc.vector.reciprocal_approx_fast(rinv[:], pl[:])
                    pending_norm.append((h, jsl, rinv))

                # Heads run in pairs: both heads' S tiles for a k-tile share
                # one 2-bank psum tile and a single exp instruction, halving
                # the ACT instruction count in the exp-bound inner loop.
                for hp in range(G // 2):
                    h0 = 2 * hp
                    po = [ps_o.tile([P, TC], F32, tag="o", name="po")
                          for _ in range(2)]
                    pipe = []

                    def drain():
                        ppo, ppt, pkt, pqs = pipe.pop(0)
                        nc.tensor.matmul(ppo[:, pqs], v_sb[:, pkt],
                                         ppt[:, pqs],
                                         start=(pkt == 0), stop=(pkt == nk - 1))

                    for kt in range(nk):
                        pace()
                        # Fire deferred denominator/normalize steps before
                        # this kt's emissions so their DVE/gpsimd chains get
                        # early queue slots and never gate the a-ring.
                        if kt in (1, 5) and pending_ones:
                            do_ones()
                        if kt in (3, 7) and pending_norm:
                            do_norm()
                        it += 1
                        m = kt - 4 * j
                        off = 0 if m < 0 else OFFS[m]
                        qs = slice(off, TC)
                        pss = ps_s.tile([P, 2, TC], F32, tag="s", name="pss")
                        for i in (0, 1):
                            nc.tensor.matmul(pss[:, i, qs],
                                             kt_sb[:, kt * P:(kt + 1) * P],
                                             qt[:, h0 + i, qs],
                                             start=True, stop=True)
                        pt = wkp.tile([P, 2, TC], BF16, tag="pt", name="pt")
                        nc.scalar.activation(pt[:, :, qs], pss[:, :, qs], EXP,
                                             scale=SCALE)
                        for i in (0, 1):
                            h = h0 + i
                            if m >= 0:
                                ssl = slice(off, off + P)
                                nc.vector.tensor_mul(out=pt[:, i, ssl],
                                                     in0=pt[:, i, ssl],
                                                     in1=mask_sb[:, m, ssl])
                            if kt == 0:
                                nc.vector.tensor_copy(acc_sb[:, h],
                                                      pt[:, i])
                            else:
                                nc.vector.tensor_add(out=acc_sb[:, h, qs],
                                                     in0=acc_sb[:, h, qs],
                                                     in1=pt[:, i, qs])
                            pipe.append((po[i], pt[:, i], kt, qs))
                            if len(pipe) > DEPTH:
                                drain()
                    while pipe:
                        drain()
                    for i in (0, 1):
                        # Evict the unnormalized PV sum; frees the psum bank
                        # without waiting for the denominator chain.
                        nc.vector.tensor_copy(otn[:, h0 + i, jsl], po[i][:])
                        pending_ones.append(h0 + i)
                while pending_ones:
                    do_ones()
                for f in filler[filled:]:
                    f()

            for f in a_thunks(0):
                f()
            for j in range(NJ):
                filler = a_thunks(j + 1) if j + 1 < NJ else []
                if j > 0:
                    filler = filler + [
                        (lambda dt=dt: c_group(j - 1, dt)) for dt in range(DT)
                    ]
                b_phase(j, filler)
                while pending_norm:
                    do_norm()
            for dt in range(DT):
                c_group(NJ - 1, dt)

    nc.compile()
    return nc


def _host_shards(inputs):
    bf16 = ml_dtypes.bfloat16
    x = np.asarray(inputs["x"], dtype=np.float32)
    cos = np.asarray(inputs["cos"], dtype=np.float32)
    sin = np.asarray(inputs["sin"], dtype=np.float32)
    Wq = np.asarray(inputs["Wq"], dtype=np.float32)
    Wk = np.asarray(inputs["Wk"], dtype=np.float32)
    Wv = np.asarray(inputs["Wv"], dtype=np.float32)
    Wo = np.asarray(inputs["Wo"], dtype=np.float32)

    cosT = np.ascontiguousarray(cos.T).astype(bf16)
    sinT = np.ascontiguousarray(sin.T).astype(bf16)
    rmat = np.zeros((HD, HD), np.float32)
    half = HD // 2
    for i in range(half):
        rmat[i + half, i] = -1.0     # out[m<64] = -q[m+64]
        rmat[i, i + half] = 1.0      # out[m>=64] = q[m-64]
    rmat = rmat.astype(bf16)
    iden = np.eye(P, dtype=np.float32).astype(bf16)
    kk = np.arange(P)[:, None, None]
    mm = np.arange(G)[None, :, None]
    qq = np.arange(TC)[None, None, :]
    masks = (qq >= kk + P * mm).astype(np.float32).astype(bf16)
    ones_k = np.ones((P, 1), np.float16)

    def to_sbuf_layout(wT, cols):
        # [D_contract, cols] -> [P, D_contract//P, cols], partition dim first
        return np.ascontiguousarray(
            wT.reshape(-1, P, cols).transpose(1, 0, 2)).astype(bf16)

    # x[b].T is [d, t]; device layout [p, j, q, dtq, t'] with d = (4q+dtq)*P+p
    # and t = j*TC + t' makes each (j, q) quarter-load fully contiguous.
    xTs = [np.ascontiguousarray(
        x[b].T.reshape(4, 4, P, NJ, TC).transpose(2, 3, 0, 1, 4)).astype(bf16)
        for b in range(B)]
    def wq_shard(kv):
        # [P, G, DT, HD]: per-head-major so each head's weights are one
        # contiguous DMA.
        heads = [to_sbuf_layout(
            Wq[kv * EQ + h * HD:kv * EQ + (h + 1) * HD].T, HD)
            for h in range(G)]
        return np.ascontiguousarray(np.stack(heads, axis=1))

    wqTs = [wq_shard(kv) for kv in range(HKV)]
    wkTs = [to_sbuf_layout(Wk[kv * HD:(kv + 1) * HD].T, HD) for kv in range(HKV)]
    wvTs = [to_sbuf_layout(Wv[kv * HD:(kv + 1) * HD].T, HD) for kv in range(HKV)]
    woTs = [to_sbuf_layout(Wo[:, kv * EQ:(kv + 1) * EQ].T, D) for kv in range(HKV)]

    in_maps = []
    for c in range(8):
        b, kv = divmod(c, HKV)
        in_maps.append({
            "xT": xTs[b], "wqT": wqTs[kv], "wkT": wkTs[kv], "wvT": wvTs[kv],
            "woT": woTs[kv], "cosT": cosT, "sinT": sinT, "rmat": rmat,
            "iden": iden, "masks": masks, "ones_k": ones_k,
        })
    return in_maps


def get_nc():
    if "nc" not in _CACHE:
        _CACHE["nc"] = _build()
    return _CACHE["nc"]


def run(inputs, **kw):
    nc = get_nc()
    in_maps = _host_shards(inputs)
    res = run_bass_kernel_spmd(nc, in_maps, core_ids=list(range(8)), **kw)
    out = np.zeros((B, T, D), np.float32)
    for c in range(8):
        b = c // HKV
        yT = res.results[c]["yT"].astype(np.float32)  # [NJ, D, TC]
        for j in range(NJ):
            out[b, j * TC:(j + 1) * TC] += yT[j].T
    return out, res


def kernel(**inputs) -> np.ndarray:
    out, _ = run(inputs)
    return out


# revision 65
# speedup vs baseline: 1.0006x; 1.0006x over previous
"""GQA attention kernel for 8 TRN2 NeuronCores (Bass/Tile, SPMD).

Sharding: core c -> (batch b = c // 4, kv-head kv = c % 4). Each core computes
the 4 query heads of its kv group for its batch and a partial (transposed)
output projection; the host sums the 4 partials per batch.

All matmul inputs are bf16 (the PE streams 1 col/cycle at any free size,
DMA and LDWEIGHTS bytes halve, and everything stays SBUF-resident — no pool
juggling). Softmax denominators accumulate on the vector engine in fp16
(acc += exp tile) with a single ones-matmul per head instead of a
per-k-tile ones-matmul chain; PV sums are evicted unnormalized and scaled
in place once the broadcast 1/l lands. Heads run in pairs sharing one
2-bank psum tile and a single exp instruction per k-tile, halving ACT
instruction count in the exp-bound inner loop. The whole kernel is software
pipelined: the next chunk's projection chains and the previous chunk's
output-projection groups are paced evenly through the attention kt-loop, so
the PE always has matmul work while the ACT engine drains the exp backlog,
and output DMA spreads across the run instead of piling up in a tail.
"""

import os
import sys

import numpy as np

for _p in ("/opt/trn_rl_repo", "/root/.axon_site/_ro/trn_rl_repo"):
    if os.path.isdir(_p) and _p not in sys.path:
        sys.path.insert(0, _p)

import ml_dtypes  # noqa: E402

import concourse.bass as bass  # noqa: E402
import concourse.mybir as mybir  # noqa: E402
from concourse import bacc  # noqa: E402
from concourse.tile import TileContext  # noqa: E402
from concourse.bass_utils import run_bass_kernel_spmd  # noqa: E402

B, T, D = 2, 2048, 2048
H, HKV, HD = 16, 4, 128
G = H // HKV            # query heads per kv head (= per core)
EQ = G * HD             # 512: query-projection rows per core
P = 128
TC = 512                # t-chunk (free dim of most matmuls)
NJ = T // TC            # 4 chunks
DT = D // P             # 16 contraction tiles
SCALE = 1.0 / float(np.sqrt(HD))

F32 = mybir.dt.float32
F32R = mybir.dt.float32r
BF16 = mybir.dt.bfloat16
F16 = mybir.dt.float16
EXP = mybir.ActivationFunctionType.Exp

_CACHE = {}


def _build():
    nc = bacc.Bacc("TRN2", target_bir_lowering=False, debug=False)

    # All inputs arrive pre-transposed into SBUF layout (partition dim first,
    # contiguous per partition) so every DMA runs at full descriptor rate.
    xT = nc.declare_dram_parameter("xT", [P, NJ, 4, 4, TC], BF16, isOutput=False)
    wqT = nc.declare_dram_parameter("wqT", [P, G, DT, HD], BF16, isOutput=False)
    wkT = nc.declare_dram_parameter("wkT", [P, DT, HD], BF16, isOutput=False)
    wvT = nc.declare_dram_parameter("wvT", [P, DT, HD], BF16, isOutput=False)
    woT = nc.declare_dram_parameter("woT", [P, G, D], BF16, isOutput=False)
    cosT = nc.declare_dram_parameter("cosT", [HD, T], BF16, isOutput=False)
    sinT = nc.declare_dram_parameter("sinT", [HD, T], BF16, isOutput=False)
    rmat = nc.declare_dram_parameter("rmat", [HD, HD], BF16, isOutput=False)
    iden = nc.declare_dram_parameter("iden", [P, P], BF16, isOutput=False)
    masks = nc.declare_dram_parameter("masks", [P, G, TC], BF16, isOutput=False)
    ones_k = nc.declare_dram_parameter("ones_k", [P, 1], F16, isOutput=False)
    yT = nc.declare_dram_parameter("yT", [NJ, D, TC], BF16, isOutput=True)

    with TileContext(nc) as tc:
        with (
            tc.tile_pool(name="const", bufs=1) as cst,
            tc.tile_pool(name="wts", bufs=1) as wts,
            tc.tile_pool(name="xs", bufs=1) as xs,
            tc.tile_pool(name="kv", bufs=1) as kvp,
            tc.tile_pool(name="qk", bufs=2) as qk,
            tc.tile_pool(name="vt", bufs=2) as vtp,
            tc.tile_pool(name="rtmp", bufs=2) as rtmp,
            tc.tile_pool(name="work", bufs=8) as wkp,
            tc.tile_pool(name="small", bufs=2) as sml,
            tc.tile_pool(name="yout", bufs=4) as yop,
            tc.tile_pool(name="ps_o", bufs=2, space="PSUM") as ps_o,
            tc.tile_pool(name="ps_s", bufs=2, space="PSUM") as ps_s,
            tc.tile_pool(name="ps_a", bufs=2, space="PSUM") as ps_a,
        ):
            # Constants ride the gpsimd SWDGE ring so they don't delay the
            # weight/x loads on the two HWDGE rings.
            cos_sb = cst.tile([HD, T], BF16, tag="cos")
            sin_sb = cst.tile([HD, T], BF16, tag="sin")
            rmat_sb = cst.tile([HD, HD], BF16, tag="rmat")
            iden_sb = cst.tile([P, P], BF16, tag="iden")
            mask_sb = cst.tile([P, G, TC], BF16, tag="mask")
            onek_sb = cst.tile([P, 1], F16, tag="onek")
            nc.gpsimd.dma_start(cos_sb[:], cosT[:])
            nc.gpsimd.dma_start(sin_sb[:], sinT[:])
            nc.gpsimd.dma_start(rmat_sb[:], rmat[:])
            nc.gpsimd.dma_start(iden_sb[:], iden[:])
            nc.gpsimd.dma_start(mask_sb[:], masks[:])
            nc.gpsimd.dma_start(onek_sb[:], ones_k[:])

            # Weights ride the scalar HWDGE ring in first-use order; x rides
            # the sync ring, quarters in consumption order. Everything is
            # resident for the whole kernel (bf16 halves the footprint).
            wq_sb = wts.tile([P, G, DT, HD], BF16, tag="wq")
            wk_sb = wts.tile([P, DT, HD], BF16, tag="wk")
            wv_sb = wts.tile([P, DT, HD], BF16, tag="wv")
            wo_sb = wts.tile([P, G, D], BF16, tag="wo")
            xq_sb = {}
            for j in range(NJ):
                for q in range(4):
                    xq_sb[(j, q)] = xs.tile([P, 4, TC], BF16,
                                            tag=f"x{j}q{q}", name=f"x{j}q{q}")

            # Both HWDGE rings share one per-core HBM bandwidth pool, so the
            # load order across BOTH rings must match consumption order:
            # (wv slice, its x j0 dt-slices) pairs, wk, per-head wq — and
            # only then the later chunks' x and wo, which otherwise steal
            # bandwidth from the weight stream exactly while the Q chains
            # starve. Slices alternate rings to keep delivery in order.
            seq = []
            for q in range(4):
                seq.append((wv_sb[:, 4 * q:4 * q + 4],
                            wvT[:, 4 * q:4 * q + 4]))
                for dq in range(4):
                    seq.append((xq_sb[(0, q)][:, dq:dq + 1],
                                xT[:, 0, q, dq:dq + 1]))
            for q in range(4):
                seq.append((wk_sb[:, 4 * q:4 * q + 4],
                            wkT[:, 4 * q:4 * q + 4]))
            for g in range(G):
                seq.append((wq_sb[:, g], wqT[:, g]))
            for q in range(4):
                # x j1 lands right as B_0's striped A_1 chains consume it;
                # per-dt slices keep them paced by arrival, not completion.
                for dq in range(4):
                    seq.append((xq_sb[(1, q)][:, dq:dq + 1],
                                xT[:, 1, q, dq:dq + 1]))
            for j in range(2, NJ):
                for q in range(4):
                    seq.append((xq_sb[(j, q)][:], xT[:, j, q]))
            for g in range(G):
                seq.append((wo_sb[:, g], woT[:, g]))
            for i, (dst, srcp) in enumerate(seq):
                (nc.sync if i % 2 == 0 else nc.scalar).dma_start(dst, srcp)

            kt_sb = kvp.tile([HD, T], BF16, tag="kt")
            v_sb = kvp.tile([P, DT, HD], BF16, tag="v")
            otn = kvp.tile([HD, G, T], BF16, tag="otn")
            acc_sb = kvp.tile([P, G, TC], F16, tag="acc")

            OFFS = {0: 0, 1: 128, 2: 256, 3: 384}
            DEPTH = 4  # exp/mask run four S-tiles ahead of PV

            qt_sb = {}

            def a_thunks(j):
                """Projections of t-chunk j + RoPE + V transpose, as a list
                of emission thunks (one per chain + a flush) so they can be
                striped into the previous attention block. Each chain's RoPE
                matmul is emitted one chain later so its eviction + cos-mul
                hide under the next chain's matmuls."""
                jsl = slice(j * TC, (j + 1) * TC)
                qt = qk.tile([HD, G, TC], BF16, tag="qt", name="qt")
                qt_sb[j] = qt
                vt = vtp.tile([HD, TC], BF16, tag="vt", name="vt")
                rope_q = []

                def finish_rope(s, t1):
                    # s <- s*cos + rotate_half(s)*sin; t1 = s*cos precomputed
                    pr = ps_s.tile([HD, TC], F32, tag="s", name="pr")
                    nc.tensor.matmul(pr[:], rmat_sb[:], s, start=True,
                                     stop=True)
                    nc.vector.tensor_mul(out=s, in0=pr[:], in1=sin_sb[:, jsl])
                    nc.vector.tensor_add(out=s, in0=s, in1=t1[:])

                def chain(a):
                    acc = ps_a.tile([P, TC], F32, tag="a", name="acc")
                    for dt in range(DT):
                        if a == 0:
                            lhsT = wv_sb[:, dt]
                        elif a == 1:
                            lhsT = wk_sb[:, dt]
                        else:
                            lhsT = wq_sb[:, a - 2, dt]
                        nc.tensor.matmul(acc[:], lhsT,
                                         xq_sb[(j, dt // 4)][:, dt % 4],
                                         start=(dt == 0), stop=(dt == DT - 1))
                    if a == 0:
                        nc.vector.tensor_copy(vt[:], acc[:])
                    else:
                        s = kt_sb[:, jsl] if a == 1 else qt[:, a - 2]
                        nc.vector.tensor_copy(s, acc[:])
                        t1 = rtmp.tile([HD, TC], BF16, tag="t1", name="t1")
                        nc.vector.tensor_mul(out=t1[:], in0=s,
                                             in1=cos_sb[:, jsl])
                        rope_q.append((s, t1))
                    if a == 1:
                        # V transpose rides the PE while K's eviction drains.
                        for tt in range(NJ):
                            pvt = ps_s.tile([P, P], BF16, tag="s", name="pvt")
                            nc.tensor.transpose(pvt[:],
                                                vt[:, tt * P:(tt + 1) * P],
                                                iden_sb[:])
                            nc.vector.tensor_copy(v_sb[:, NJ * j + tt],
                                                  pvt[:])
                    if len(rope_q) >= 2:
                        finish_rope(*rope_q.pop(0))

                def flush():
                    while rope_q:
                        finish_rope(*rope_q.pop(0))

                return [lambda a=a: chain(a) for a in range(6)] + [flush]

            pending_norm = []

            def do_norm():
                # otn holds the unnormalized PV sum; scale it by 1/l in
                # place once the broadcast lands.
                h, jsl, rinv = pending_norm.pop(0)
                binv = sml.tile([P, TC], F32, tag="binv", name="binv")
                nc.gpsimd.partition_broadcast(binv[:], rinv[:])
                nc.vector.tensor_mul(out=otn[:, h, jsl], in0=otn[:, h, jsl],
                                     in1=binv[:])

            def c_group(j, dt):
                """One output-projection dt-group for t-chunk j; copies
                alternate DVE/ACT and DMA issues alternate sync/scalar so no
                single engine paces the chain."""
                jsl = slice(j * TC, (j + 1) * TC)
                py = ps_a.tile([P, TC], F32, tag="a", name="py")
                for g in range(G):
                    nc.tensor.matmul(py[:],
                                     wo_sb[:, g, dt * P:(dt + 1) * P],
                                     otn[:, g, jsl],
                                     start=(g == 0), stop=(g == G - 1))
                y_sb = yop.tile([P, TC], BF16, tag="ysb", name="ysb")
                if dt % 2 == 0:
                    nc.scalar.copy(y_sb[:], py[:])
                    nc.sync.dma_start(yT[j, dt * P:(dt + 1) * P, :], y_sb[:])
                else:
                    nc.vector.tensor_copy(y_sb[:], py[:])
                    nc.scalar.dma_start(yT[j, dt * P:(dt + 1) * P, :],
                                        y_sb[:])

            def b_phase(j, filler):
                """Attention for q-block j, all 4 heads, causal. `filler` is
                a list of emission thunks (next chunk's projections, previous
                chunk's output groups) paced evenly through the kt loop so
                the PE always has matmul work while the ACT engine drains the
                exp backlog."""
                jsl = slice(j * TC, (j + 1) * TC)
                qt = qt_sb[j]
                nk = 4 * (j + 1)
                n_it = (G // 2) * nk
                it = 0
                filled = 0

                def pace():
                    nonlocal filled
                    want = (len(filler) * it) // n_it
                    while filled < want:
                        filler[filled]()
                        filled += 1

                pending_ones = []

                def do_ones():
                    # Softmax denominator: one ones-matmul over the DVE-
                    # accumulated exp sums. Deferred into the next head pair
                    # so the PE isn't stalled on the exp->accumulate chain
                    # at the boundary.
                    h = pending_ones.pop(0)
                    pl = ps_a.tile([1, TC], F32, tag="a", name="pl")
                    nc.tensor.matmul(pl[:], onek_sb[:], acc_sb[:, h],
                                     start=True, stop=True)
                    rinv = sml.tile([1, TC], F32, tag="rinv", name="rinv")
                    nc.vector.reciprocal_approx_fast(rinv[:], pl[:])
                    pending_norm.append((h, jsl, rinv))

                # Heads run in pairs: both heads' S tiles for a k-tile share
                # one 2-bank psum tile and a single exp instruction, halving
                # the ACT instruction count in the exp-bound inner loop.
                for hp in range(G // 2):
                    h0 = 2 * hp
                    po = [ps_o.tile([P, TC], F32, tag="o", name="po")
                          for _ in range(2)]
                    pipe = []

                    def drain():
                        ppo, ppt, pkt, pqs = pipe.pop(0)
                        nc.tensor.matmul(ppo[:, pqs], v_sb[:, pkt],
                                         ppt[:, pqs],
                                         start=(pkt == 0), stop=(pkt == nk - 1))

                    for kt in range(nk):
                        pace()
                        it += 1
                        m = kt - 4 * j
                        off = 0 if m < 0 else OFFS[m]
                        qs = slice(off, TC)
                        pss = ps_s.tile([P, 2, TC], F32, tag="s", name="pss")
                        for i in (0, 1):
                            nc.tensor.matmul(pss[:, i, qs],
                                             kt_sb[:, kt * P:(kt + 1) * P],
                                             qt[:, h0 + i, qs],
                                             start=True, stop=True)
                        pt = wkp.tile([P, 2, TC], BF16, tag="pt", name="pt")
                        nc.scalar.activation(pt[:, :, qs], pss[:, :, qs], EXP,
                                             scale=SCALE)
                        for i in (0, 1):
                            h = h0 + i
                            if m >= 0:
                                ssl = slice(off, off + P)
                                nc.vector.tensor_mul(out=pt[:, i, ssl],
                                                     in0=pt[:, i, ssl],
                                                     in1=mask_sb[:, m, ssl])
                            if kt == 0:
                                nc.vector.tensor_copy(acc_sb[:, h],
                                                      pt[:, i])
                            else:
                                nc.vector.tensor_add(out=acc_sb[:, h, qs],
                                                     in0=acc_sb[:, h, qs],
                                                     in1=pt[:, i, qs])
                            pipe.append((po[i], pt[:, i], kt, qs))
                            if len(pipe) > DEPTH:
                                drain()
                        if kt == 1 and pending_ones:
                            do_ones()
                        if kt == 3 and pending_norm:
                            do_norm()
                        if kt == 5 and pending_ones:
                            do_ones()
                        if kt == 7 and pending_norm:
                            do_norm()
                    while pipe:
                        drain()
                    for i in (0, 1):
                        # Evict the unnormalized PV sum; frees the psum bank
                        # without waiting for the denominator chain.
                        nc.vector.tensor_copy(otn[:, h0 + i, jsl], po[i][:])
                        pending_ones.append(h0 + i)
                while pending_ones:
                    do_ones()
                for f in filler[filled:]:
                    f()

            for f in a_thunks(0):
                f()
            for j in range(NJ):
                filler = a_thunks(j + 1) if j + 1 < NJ else []
                if j > 0:
                    filler = filler + [
                        (lambda dt=dt: c_group(j - 1, dt)) for dt in range(DT)
                    ]
                b_phase(j, filler)
                while pending_norm:
                    do_norm()
            for dt in range(DT):
                c_group(NJ - 1, dt)

    nc.compile()
    return nc


def _host_shards(inputs):
    bf16 = ml_dtypes.bfloat16
    x = np.asarray(inputs["x"], dtype=np.float32)
    cos = np.asarray(inputs["cos"], dtype=np.float32)
    sin = np.asarray(inputs["sin"], dtype=np.float32)
    Wq = np.asarray(inputs["Wq"], dtype=np.float32)
    Wk = np.asarray(inputs["Wk"], dtype=np.float32)
    Wv = np.asarray(inputs["Wv"], dtype=np.float32)
    Wo = np.asarray(inputs["Wo"], dtype=np.float32)

    cosT = np.ascontiguousarray(cos.T).astype(bf16)
    sinT = np.ascontiguousarray(sin.T).astype(bf16)
    rmat = np.zeros((HD, HD), np.float32)
    half = HD // 2
    for i in range(half):
        rmat[i + half, i] = -1.0     # out[m<64] = -q[m+64]
        rmat[i, i + half] = 1.0      # out[m>=64] = q[m-64]
    rmat = rmat.astype(bf16)
    iden = np.eye(P, dtype=np.float32).astype(bf16)
    kk = np.arange(P)[:, None, None]
    mm = np.arange(G)[None, :, None]
    qq = np.arange(TC)[None, None, :]
    masks = (qq >= kk + P * mm).astype(np.float32).astype(bf16)
    ones_k = np.ones((P, 1), np.float16)

    def to_sbuf_layout(wT, cols):
        # [D_contract, cols] -> [P, D_contract//P, cols], partition dim first
        return np.ascontiguousarray(
            wT.reshape(-1, P, cols).transpose(1, 0, 2)).astype(bf16)

    # x[b].T is [d, t]; device layout [p, j, q, dtq, t'] with d = (4q+dtq)*P+p
    # and t = j*TC + t' makes each (j, q) quarter-load fully contiguous.
    xTs = [np.ascontiguousarray(
        x[b].T.reshape(4, 4, P, NJ, TC).transpose(2, 3, 0, 1, 4)).astype(bf16)
        for b in range(B)]
    def wq_shard(kv):
        # [P, G, DT, HD]: per-head-major so each head's weights are one
        # contiguous DMA.
        heads = [to_sbuf_layout(
            Wq[kv * EQ + h * HD:kv * EQ + (h + 1) * HD].T, HD)
            for h in range(G)]
        return np.ascontiguousarray(np.stack(heads, axis=1))

    wqTs = [wq_shard(kv) for kv in range(HKV)]
    wkTs = [to_sbuf_layout(Wk[kv * HD:(kv + 1) * HD].T, HD) for kv in range(HKV)]
    wvTs = [to_sbuf_layout(Wv[kv * HD:(kv + 1) * HD].T, HD) for kv in range(HKV)]
    woTs = [to_sbuf_layout(Wo[:, kv * EQ:(kv + 1) * EQ].T, D) for kv in range(HKV)]

    in_maps = []
    for c in range(8):
        b, kv = divmod(c, HKV)
        in_maps.append({
            "xT": xTs[b], "wqT": wqTs[kv], "wkT": wkTs[kv], "wvT": wvTs[kv],
            "woT": woTs[kv], "cosT": cosT, "sinT": sinT, "rmat": rmat,
            "iden": iden, "masks": masks, "ones_k": ones_k,
        })
    return in_maps


def get_nc():
    if "nc" not in _CACHE:
        _CACHE["nc"] = _build()
    return _CACHE["nc"]


def run(inputs, **kw):
    nc = get_nc()
    in_maps = _host_shards(inputs)
    res = run_bass_kernel_spmd(nc, in_maps, core_ids=list(range(8)), **kw)
    out = np.zeros((B, T, D), np.float32)
    for c in range(8):
        b = c // HKV
        yT = res.results[c]["yT"].astype(np.float32)  # [NJ, D, TC]
        for j in range(NJ):
            out[b, j * TC:(j + 1) * TC] += yT[j].T
    return out, res


def kernel(**inputs) -> np.ndarray:
    out, _ = run(inputs)
    return out
